# revision 2
# baseline (speedup 1.0000x reference)
"""Trainium2 Bass kernel for per-sample 2-expert MoE residual MLP.

Reference computation (per sample b, expert e = cond[b]):
    h = relu(Wd[e] @ x_b + bd[e])        # [MID, H*W]
    y = Wu[e] @ h + bu[e] + x_b          # [C, H*W]

Shapes: x [8, 1024, 64, 64] f32, Wd [2, 256, 1024], bd [2, 256],
        Wu [2, 1024, 256], bu [2, 1024], cond [8] int.

Sharding: data-parallel over batch — one sample per NeuronCore (8 cores).
The expert gather (Wd[cond[b]]) happens on host while building each
core's input map.

On-device per core, pipelined over 8 spatial blocks of 512 columns:
  DMA x block (fp32) -> cast to bf16 (DVE) -> GEMM1 (bf16 PE, fp32 PSUM)
  -> bias+ReLU+cast-to-bf16 (ACT) -> GEMM2 (bf16) -> bias (ACT, fp32)
  -> + x residual in fp32 (DVE) -> DMA out.
The residual path stays fully fp32; only the two GEMM multiplies are
bf16 (error lands ~2e-4 absolute on y whose scale is ~1).
"""

import numpy as np
import ml_dtypes
from contextlib import ExitStack

import concourse.bacc as bacc
import concourse.mybir as mybir
import concourse.tile as tile
from concourse.bass_utils import run_bass_kernel_spmd

# Problem dims (hardcoded per contract).
B = 8
C = 1024
MID = 256
H = 64
W = 64
HW = H * W  # 4096

P = 128            # partitions
NB = 512           # spatial columns per block (one PSUM bank of fp32)
N_BLOCKS = HW // NB  # 8
KC = C // P        # 8  k-tiles for GEMM1 / m-tiles for GEMM2
KM = MID // P      # 2  m-tiles for GEMM1 / k-tiles for GEMM2

F32 = mybir.dt.float32
BF16 = mybir.dt.bfloat16


def build_nc(x_bufs=3, psum_h_bufs=2, psum_y_bufs=4, y_bufs=4):
    """Build the per-core Bass program (SPMD: same program on all cores)."""
    nc = bacc.Bacc("TRN2", target_bir_lowering=False, debug=False)

    x_d = nc.dram_tensor("x", [C, HW], F32, kind="ExternalInput")
    wdT_d = nc.dram_tensor("wdT", [C, MID], BF16, kind="ExternalInput")
    wuT_d = nc.dram_tensor("wuT", [MID, C], BF16, kind="ExternalInput")
    bd_d = nc.dram_tensor("bd", [P, KM], F32, kind="ExternalInput")
    bu_d = nc.dram_tensor("bu", [P, KC], F32, kind="ExternalInput")
    y_d = nc.dram_tensor("y", [C, HW], F32, kind="ExternalOutput")

    with tile.TileContext(nc) as tc, ExitStack() as ctx:
        wpool = ctx.enter_context(tc.tile_pool(name="w", bufs=1))
        xpool = ctx.enter_context(tc.tile_pool(name="xp", bufs=x_bufs))
        xbpool = ctx.enter_context(tc.tile_pool(name="xbp", bufs=2))
        hpool = ctx.enter_context(tc.tile_pool(name="hp", bufs=2))
        ypool = ctx.enter_context(tc.tile_pool(name="yp", bufs=y_bufs))
        psh = ctx.enter_context(tc.tile_pool(name="ph", bufs=psum_h_bufs, space="PSUM"))
        psy = ctx.enter_context(tc.tile_pool(name="py", bufs=psum_y_bufs, space="PSUM"))

        # Resident weights and biases.
        wd_s = wpool.tile([P, KC, MID], BF16, tag="wd")
        for k in range(KC):
            nc.sync.dma_start(wd_s[:, k, :], wdT_d[k * P:(k + 1) * P, :])
        wu_s = wpool.tile([P, KM, C], BF16, tag="wu")
        for k in range(KM):
            nc.sync.dma_start(wu_s[:, k, :], wuT_d[k * P:(k + 1) * P, :])
        bd_s = wpool.tile([P, KM], F32, tag="bd")
        nc.sync.dma_start(bd_s[:], bd_d[:])
        bu_s = wpool.tile([P, KC], F32, tag="bu")
        nc.sync.dma_start(bu_s[:], bu_d[:])

        for n in range(N_BLOCKS):
            xt = xpool.tile([P, KC, NB], F32, tag="xt")
            for k in range(KC):
                nc.sync.dma_start(xt[:, k, :], x_d[k * P:(k + 1) * P, n * NB:(n + 1) * NB])

            # Cast x block to bf16 for the GEMM1 moving operand.
            xb = xbpool.tile([P, KC, NB], BF16, tag="xb")
            for k in range(KC):
                nc.vector.tensor_copy(xb[:, k, :], xt[:, k, :])

            # GEMM1: h[m] = relu(sum_k wd[k,m].T @ x[k] + bd[m]), cast bf16
            ht = hpool.tile([P, KM, NB], BF16, tag="ht")
            for m in range(KM):
                ph = psh.tile([P, NB], F32, tag="ph")
                for k in range(KC):
                    nc.tensor.matmul(
                        ph[:],
                        wd_s[:, k, m * P:(m + 1) * P],
                        xb[:, k, :],
                        start=(k == 0),
                        stop=(k == KC - 1),
                    )
                nc.scalar.activation(
                    ht[:, m, :], ph[:],
                    mybir.ActivationFunctionType.Relu,
                    bias=bd_s[:, m:m + 1],
                )

            # GEMM2: y[mc] = sum_km wu[km,mc].T @ h[km] + bu[mc] + x[mc]
            for mc in range(KC):
                py = psy.tile([P, NB], F32, tag="py")
                for km in range(KM):
                    nc.tensor.matmul(
                        py[:],
                        wu_s[:, km, mc * P:(mc + 1) * P],
                        ht[:, km, :],
                        start=(km == 0),
                        stop=(km == KM - 1),
                    )
                yt = ypool.tile([P, NB], F32, tag="yt")
                nc.scalar.activation(
                    yt[:], py[:],
                    mybir.ActivationFunctionType.Identity,
                    bias=bu_s[:, mc:mc + 1],
                )
                nc.vector.tensor_add(yt[:], yt[:], xt[:, mc, :])
                nc.sync.dma_start(y_d[mc * P:(mc + 1) * P, n * NB:(n + 1) * NB], yt[:])

    nc.compile()
    return nc


_NC = None


def get_nc():
    global _NC
    if _NC is None:
        _NC = build_nc()
    return _NC


def make_in_maps(inputs):
    x = np.asarray(inputs["x"], dtype=np.float32)
    Wd = np.asarray(inputs["Wd"], dtype=np.float32)
    bd = np.asarray(inputs["bd"], dtype=np.float32)
    Wu = np.asarray(inputs["Wu"], dtype=np.float32)
    bu = np.asarray(inputs["bu"], dtype=np.float32)
    cond = np.asarray(inputs["cond"]).astype(np.int64)

    in_maps = []
    for b in range(B):
        e = int(cond[b])
        in_maps.append({
            "x": np.ascontiguousarray(x[b].reshape(C, HW)),
            "wdT": np.ascontiguousarray(Wd[e].T).astype(ml_dtypes.bfloat16),  # [C, MID]
            "wuT": np.ascontiguousarray(Wu[e].T).astype(ml_dtypes.bfloat16),  # [MID, C]
            "bd": np.ascontiguousarray(bd[e].reshape(KM, P).T),  # [P, KM]
            "bu": np.ascontiguousarray(bu[e].reshape(KC, P).T),  # [P, KC]
        })
    return in_maps


def run_sharded(inputs, **kwargs):
    """Run on all 8 cores; returns (stacked output [B,C,H,W], BassKernelResults)."""
    nc = get_nc()
    in_maps = make_in_maps(inputs)
    res = run_bass_kernel_spmd(nc, in_maps, core_ids=list(range(B)), **kwargs)
    out = np.stack([res.results[b]["y"].reshape(C, H, W) for b in range(B)])
    return out, res


def kernel(**inputs) -> np.ndarray:
    out, _ = run_sharded(inputs)
    return out
